# revision 6
# baseline (speedup 1.0000x reference)
"""Causal self-attention (T=2048, C=1024, H=16) on 8 Trainium2 NeuronCores.

Tensor-parallel over heads: each core owns 2 heads (wqkv row-shard), computes
qkv + attention for its heads, then multiplies attention rows by the full
projection matrix to produce a PARTIAL token-major output. The host sums the
8 partials and adds (b_v @ proj_w.T + proj_b) once — v-bias commutes with
softmax-normalized attention, so both biases fold into the host-side reduce.

v4 (vs v3 at ~104-107us):
  - no dummy-warmup phase beyond ~8 matmuls: the input DMA order is arranged
    so qkv(0) itself starts ~3us after the framework preamble and doubles as
    the HAM warm-up.
  - transposed proj: lhsT = attn token-tile, rhs = proj rows -> psum is
    [128 tokens, 512 outs] and the output DMA is token-major [2048, 1024]
    fp16 with 2KB contiguous lines; no host transpose.
  - normalize rescheduled off the PE critical path: denominator row -> ACT
    copy -> f32r ones-broadcast matmul (borrowing a scores psum bank) ->
    DVE reciprocal -> DVE multiply, all emitted between the leftover qkv
    fillers so the PE FIFO never stalls at a chunk boundary (the v3 stall
    re-engaged the HAM clock gate: 32us of the run at 1.2GHz).
  - scores per head in separate psum banks (ring of 2); the two heads' K=64
    matmuls occupy disjoint PE row groups and stream concurrently.
  - diag-block causal mask: exp is split so the masked [128,128] block is
    exp'd first and multiplied by the 0/1 mask on GpSimd while ACT exps the
    rest of the tile - the mask latency hides under the remaining exp.
  - psum: 8 banks = scores 2 + attnT accum 2 + qkv state 2 + proj 2.
  - proj psum evacuations split DVE/ACT at the tail; per-2-token-tile output
    DMAs so the final drain waits on ~256KB, not a whole chunk.
"""

from collections import deque

import numpy as np
import ml_dtypes

import concourse.bass as bass
import concourse.mybir as mybir
import concourse.tile as tile
from concourse import bacc
from concourse import bass_utils

T = 2048
C = 1024
H = 16
D = 64
N_CORES = 8
P = 128
W = 512            # query-chunk width
NG = T // W        # 4 chunks
NO = C // P        # 8 contraction subtiles
NT = T // P        # 16 token tiles

F32 = mybir.dt.float32
F32R = mybir.dt.float32r
BF16 = mybir.dt.bfloat16
F16 = mybir.dt.float16
BF16_NP = ml_dtypes.bfloat16
F16_NP = np.float16

JK, JQ, JV = 0, 1, 2   # wT column sections: k, q, v


def _build():
    nc = bacc.Bacc("TRN2", target_bir_lowering=False, debug=False,
                   num_devices=N_CORES)

    xT = nc.dram_tensor("xT", [C, T], BF16, kind="ExternalInput").ap()
    wT = nc.dram_tensor("wT", [C, 3 * P], BF16, kind="ExternalInput").ap()
    bkq = nc.dram_tensor("bkq", [P, 2], F32, kind="ExternalInput").ap()
    mask01 = nc.dram_tensor("mask01", [P, 2, P], BF16, kind="ExternalInput").ap()
    pw = nc.dram_tensor("pw", [P, C], BF16, kind="ExternalInput").ap()
    outO = nc.dram_tensor("outO", [T, C], F16, kind="ExternalOutput").ap()

    xT3 = xT.rearrange("(o p) t -> p o t", p=P)      # [128, 8, 2048]
    wT3 = wT.rearrange("(o p) j -> p o j", p=P)      # [128, 8, 384]
    out3 = outO.rearrange("(n p) o -> p n o", p=P)   # [128, 16, 1024]

    with tile.TileContext(nc) as tc:
        with (
            tc.tile_pool(name="sb", bufs=1) as sb,
            tc.tile_pool(name="ps", bufs=1, space="PSUM") as ps,
        ):
            warm = sb.tile([P, W], BF16, name="warm")
            wT_sb = sb.tile([P, NO, 3 * P], BF16, name="wt")
            x_sb = sb.tile([P, NO, T], BF16, name="x")
            bkq_sb = sb.tile([P, 2], F32, name="bkq")
            mask_sb = sb.tile([P, 2, P], BF16, name="mask")
            pw_sb = sb.tile([P, C], BF16, name="pw")
            ones_sb = sb.tile([1, D], BF16, name="ones")
            kT_sb = sb.tile([P, T], BF16, name="kt")
            qT_sb = sb.tile([P, T], BF16, name="qt")
            v_sb = sb.tile([P, NT, 2, D + 1], BF16, name="v")
            e2r = [sb.tile([P, 2, W], BF16, name=f"e2_{i}") for i in range(3)]
            rsr = [sb.tile([1, W], BF16, name=f"rs_{i}") for i in range(2)]
            rrr = [sb.tile([D, W], F32, name=f"rr_{i}") for i in range(2)]
            attnr = [sb.tile([P, W], BF16, name=f"attn_{i}") for i in range(2)]
            out_sb = sb.tile([P, 4, C], F16, name="out")

            scr = [ps.tile([P, W], F32, name=f"sc_{i}") for i in range(2)]
            atr = [ps.tile([P, W], F32, name=f"at_{i}") for i in range(2)]
            qsr = [ps.tile([P, W], F32, name=f"qs_{i}") for i in range(2)]
            prr = [ps.tile([P, W], F32, name=f"pr_{i}") for i in range(2)]

            nc.vector.memset(warm[:], 1.0)
            nc.vector.memset(ones_sb[:], 1.0)
            nc.vector.memset(v_sb[:, :, :, D], 1.0)

            # ---- PE warm-up on the memset tile until the first x chunk
            # lands; the HAM clock gate needs ~3.4us of sustained activity ----
            for _ in range(8):
                nc.tensor.matmul(prr[0][:], lhsT=warm[:, 0:P], rhs=warm[:],
                                 start=True, stop=True)

            # ---- input DMAs: wT first on sync (needed by the first qkv
            # LDWEIGHTS), x chunk 0 first on gpsimd; small tensors next; the
            # remaining x chunks stream behind on both queues ----
            nc.sync.dma_start(wT_sb[:], wT3[:])
            nc.gpsimd.dma_start(x_sb[:, :, 0:W], xT3[:, :, 0:W])
            nc.sync.dma_start(x_sb[:, :, W:2 * W], xT3[:, :, W:2 * W])
            nc.gpsimd.dma_start(mask_sb[:], mask01)
            nc.gpsimd.dma_start(bkq_sb[:], bkq)
            nc.gpsimd.dma_start(pw_sb[:], pw)
            nc.sync.dma_start(x_sb[:, :, 2 * W:3 * W], xT3[:, :, 2 * W:3 * W])
            nc.gpsimd.dma_start(x_sb[:, :, 3 * W:4 * W], xT3[:, :, 3 * W:4 * W])

            fill_qkv = deque()
            fill_proj = deque()
            state = {"qs": 0, "pr": 0, "dmaq": 0}

            def qkv_block(b):
                """Queue qkv for token block b (512 tokens) as PE thunks."""
                cols = slice(b * W, (b + 1) * W)
                st = {}

                def kq_mm(sec, dst, bi, o):
                    def f():
                        if o == 0:
                            st[sec] = qsr[state["qs"] % 2]
                            state["qs"] += 1
                        nc.tensor.matmul(
                            st[sec][:],
                            lhsT=wT_sb[:, o, sec * P:(sec + 1) * P],
                            rhs=x_sb[:, o, cols],
                            start=(o == 0), stop=(o == NO - 1),
                        )
                        if o == NO - 1:
                            # k evac on DVE, q on ACT (per-partition bias)
                            if sec == JK:
                                nc.vector.tensor_scalar_add(
                                    dst[:, cols], st[sec][:], bkq_sb[:, 0:1])
                            else:
                                nc.scalar.add(dst[:, cols], st[sec][:],
                                              bkq_sb[:, 1:2])
                    return f

                def v_mm(tt, o):
                    def f():
                        if tt == 0 and o == 0:
                            st[JV] = qsr[state["qs"] % 2]
                            state["qs"] += 1
                        st4 = st[JV].rearrange("p (a h d) -> p a h d",
                                               a=4, h=2, d=D)
                        t0 = b * W + tt * P
                        nc.tensor.matmul(
                            st4[:, tt, :, :],
                            lhsT=x_sb[:, o, t0:t0 + P],
                            rhs=wT_sb[:, o, JV * P:(JV + 1) * P],
                            start=(o == 0), stop=(o == NO - 1),
                        )
                        if tt == 3 and o == NO - 1:
                            nc.vector.tensor_copy(
                                out=v_sb[:, 4 * b:4 * b + 4, :, 0:D],
                                in_=st4[:])
                    return f

                for sec, dst in ((JK, kT_sb), (JQ, qT_sb)):
                    for o in range(NO):
                        fill_qkv.append(kq_mm(sec, dst, sec, o))
                for tt in range(4):
                    for o in range(NO):
                        fill_qkv.append(v_mm(tt, o))

            def proj_tt(c, tt, last):
                """Project one token tile of chunk c; evacuate + DMA."""
                def f():
                    a = attnr[c % 2][:, tt * P:(tt + 1) * P]
                    p0 = prr[state["pr"] % 2]
                    p1 = prr[(state["pr"] + 1) % 2]
                    state["pr"] += 2
                    nc.tensor.matmul(p0[:], lhsT=a, rhs=pw_sb[:, 0:W],
                                     start=True, stop=True)
                    nc.tensor.matmul(p1[:], lhsT=a, rhs=pw_sb[:, W:C],
                                     start=True, stop=True)
                    if last:
                        # tail: ACT has no more exps; split the evacuation
                        nc.vector.tensor_copy(out=out_sb[:, tt, 0:W], in_=p0[:])
                        nc.scalar.copy(out_sb[:, tt, W:C], p1[:])
                    else:
                        nc.vector.tensor_copy(out=out_sb[:, tt, 0:W], in_=p0[:])
                        nc.vector.tensor_copy(out=out_sb[:, tt, W:C], in_=p1[:])
                    if tt % 2 == 1:
                        gt = c * 4 + tt
                        src = out_sb[:, tt - 1:tt + 1, :]
                        dst = out3[:, gt - 1:gt + 1, :]
                        if state["dmaq"] % 2 == 0:
                            nc.sync.dma_start(dst, src)
                        else:
                            nc.gpsimd.dma_start(dst, src)
                        state["dmaq"] += 1
                return f

            def pop(dq, n):
                for _ in range(n):
                    if dq:
                        dq.popleft()()

            def attention(c):
                t0 = c * W
                nj = 4 * (c + 1)
                keep = 2 if c == NG - 1 else 0
                pend = None

                def av(j, tq, wj, e2):
                    off = tq - t0
                    for h in range(2):
                        nc.tensor.matmul(
                            atr[h][0:D + 1, off:W],
                            lhsT=v_sb[:, j, h, :],
                            rhs=e2[:, h, 0:wj],
                            start=(j == 0), stop=(j == nj - 1),
                        )

                for j in range(nj):
                    diag = j >= 4 * c
                    tq = P * j if diag else t0
                    wj = t0 + W - tq
                    for h in range(2):
                        nc.tensor.matmul(
                            scr[h][:, 0:wj],
                            lhsT=kT_sb[h * D:(h + 1) * D, j * P:(j + 1) * P],
                            rhs=qT_sb[h * D:(h + 1) * D, tq:t0 + W],
                            start=True, stop=True,
                        )
                    pop(fill_qkv, 2)
                    if len(fill_proj) > keep:
                        pop(fill_proj, 1)
                    if pend is not None:
                        av(*pend)
                    e2 = e2r[j % 3]
                    if diag:
                        # masked diag block exp'd first; GpSimd applies the
                        # 0/1 mask while ACT exps the rest of the tile
                        for h in range(2):
                            nc.scalar.activation(
                                e2[:, h, 0:P], scr[h][:, 0:P],
                                mybir.ActivationFunctionType.Exp)
                        nc.gpsimd.tensor_mul(out=e2[:, :, 0:P],
                                             in0=e2[:, :, 0:P],
                                             in1=mask_sb[:])
                        if wj > P:
                            for h in range(2):
                                nc.scalar.activation(
                                    e2[:, h, P:wj], scr[h][:, P:wj],
                                    mybir.ActivationFunctionType.Exp)
                    else:
                        for h in range(2):
                            nc.scalar.activation(
                                e2[:, h, 0:wj], scr[h][:, 0:wj],
                                mybir.ActivationFunctionType.Exp)
                    pend = (j, tq, wj, e2)
                av(*pend)

                # ---- boundary: denominator broadcast + normalize, paced by
                # the leftover qkv fillers so the PE FIFO stays fed ----
                for h in range(2):
                    nc.scalar.copy(rsr[h][:], atr[h][D:D + 1, 0:W])
                pop(fill_qkv, 4)
                # borrow the two scores banks between chunks (one per head;
                # matmul psum output must start at partition 0)
                for h in range(2):
                    nc.tensor.matmul(
                        scr[h][0:D, :],
                        lhsT=ones_sb[:],
                        rhs=rsr[h][:],
                        start=True, stop=True,
                    )
                pop(fill_qkv, 2)
                for h in range(2):
                    nc.vector.reciprocal_approx_fast(
                        out=rrr[h][:], in_=scr[h][0:D, :])
                pop(fill_qkv, 2)
                for h in range(2):
                    nc.vector.tensor_mul(
                        out=attnr[c % 2][h * D:(h + 1) * D, :],
                        in0=atr[h][0:D, 0:W], in1=rrr[h][:])
                while fill_qkv:
                    fill_qkv.popleft()()

            # ---- main schedule ----
            qkv_block(0)
            while fill_qkv:
                fill_qkv.popleft()()
            qkv_block(1)
            for c in range(NG):
                attention(c)
                if c + 2 < NG:
                    qkv_block(c + 2)
                for tt in range(4):
                    fill_proj.append(proj_tt(c, tt, last=(c == NG - 1)))
            while fill_proj:
                fill_proj.popleft()()

    nc.compile()
    return nc


_NC = None
LAST_RESULT = None


def _get_nc():
    global _NC
    if _NC is None:
        _NC = _build()
    return _NC


def _prep_inputs(x, wqkv_w, wqkv_b, proj_w, proj_b):
    x = np.asarray(x, np.float32)
    wqkv_w = np.asarray(wqkv_w, np.float32)
    wqkv_b = np.asarray(wqkv_b, np.float32)
    proj_w = np.asarray(proj_w, np.float32)

    scale = np.float32(1.0 / np.sqrt(D))  # 0.125 exactly
    xT = np.ascontiguousarray(x.T).astype(BF16_NP)
    mask1 = np.triu(np.ones((P, P), np.float32))
    mask = np.ascontiguousarray(
        np.broadcast_to(mask1[:, None, :], (P, 2, P))).astype(BF16_NP)

    in_maps = []
    for c in range(N_CORES):
        qs = slice(P * c, P * (c + 1))
        ks = slice(C + P * c, C + P * (c + 1))
        vs = slice(2 * C + P * c, 2 * C + P * (c + 1))
        # column order in wT: k, q (pre-scaled), v
        w_c = np.concatenate(
            [wqkv_w[ks], wqkv_w[qs] * scale, wqkv_w[vs]], axis=0)  # [384, 1024]
        b_c = np.stack(
            [wqkv_b[ks], wqkv_b[qs] * scale], axis=1)              # [128, 2]
        in_maps.append({
            "xT": xT,
            "wT": np.ascontiguousarray(w_c.T).astype(BF16_NP),
            "bkq": np.ascontiguousarray(b_c, dtype=np.float32),
            "mask01": mask,
            # proj rows for this core's 128 attn dims -> [128, 1024]
            "pw": np.ascontiguousarray(proj_w[:, qs].T).astype(BF16_NP),
        })
    return in_maps


def kernel(x, wqkv_w, wqkv_b, proj_w, proj_b):
    global LAST_RESULT
    nc = _get_nc()
    in_maps = _prep_inputs(x, wqkv_w, wqkv_b, proj_w, proj_b)
    res = bass_utils.run_bass_kernel_spmd(nc, in_maps,
                                          core_ids=list(range(N_CORES)))
    LAST_RESULT = res
    # unshard: partials are sum-sharded over cores; v-bias and proj bias fold
    # into one host-side vector add (softmax rows sum to 1).
    acc = res.results[0]["outO"].astype(np.float32)
    for c in range(1, N_CORES):
        acc = acc + res.results[c]["outO"].astype(np.float32)
    b_v = np.asarray(wqkv_b, np.float32)[2 * C:3 * C]
    bias = b_v @ np.asarray(proj_w, np.float32).T + np.asarray(proj_b, np.float32)
    return np.ascontiguousarray(acc + bias[None, :]).astype(np.float32)


# revision 13
# speedup vs baseline: 1.0002x; 1.0002x over previous
"""Causal self-attention (T=2048, C=1024, H=16) on 8 Trainium2 NeuronCores.

Tensor-parallel over heads: each core owns 2 heads (wqkv row-shard), computes
qkv + attention for its heads, then multiplies attention rows by the full
projection matrix to produce a PARTIAL token-major output. The host sums the
8 partials and adds (b_v @ proj_w.T + proj_b) once — v-bias commutes with
softmax-normalized attention, so both biases fold into the host-side reduce.

v4 (vs v3 at ~104-107us):
  - no dummy-warmup phase beyond ~8 matmuls: the input DMA order is arranged
    so qkv(0) itself starts ~3us after the framework preamble and doubles as
    the HAM warm-up.
  - transposed proj: lhsT = attn token-tile, rhs = proj rows -> psum is
    [128 tokens, 512 outs] and the output DMA is token-major [2048, 1024]
    fp16 with 2KB contiguous lines; no host transpose.
  - normalize rescheduled off the PE critical path: denominator row -> ACT
    copy -> f32r ones-broadcast matmul (borrowing a scores psum bank) ->
    DVE reciprocal -> DVE multiply, all emitted between the leftover qkv
    fillers so the PE FIFO never stalls at a chunk boundary (the v3 stall
    re-engaged the HAM clock gate: 32us of the run at 1.2GHz).
  - scores per head in separate psum banks (ring of 2); the two heads' K=64
    matmuls occupy disjoint PE row groups and stream concurrently.
  - diag-block causal mask: exp is split so the masked [128,128] block is
    exp'd first and multiplied by the 0/1 mask on GpSimd while ACT exps the
    rest of the tile - the mask latency hides under the remaining exp.
  - psum: 8 banks = scores 2 + attnT accum 2 + qkv state 2 + proj 2.
  - proj psum evacuations split DVE/ACT at the tail; per-2-token-tile output
    DMAs so the final drain waits on ~256KB, not a whole chunk.
"""

from collections import deque

import numpy as np
import ml_dtypes

import concourse.bass as bass
import concourse.mybir as mybir
import concourse.tile as tile
from concourse import bacc
from concourse import bass_utils

T = 2048
C = 1024
H = 16
D = 64
N_CORES = 8
P = 128
W = 512            # query-chunk width
NG = T // W        # 4 chunks
NO = C // P        # 8 contraction subtiles
NT = T // P        # 16 token tiles

F32 = mybir.dt.float32
F32R = mybir.dt.float32r
BF16 = mybir.dt.bfloat16
F16 = mybir.dt.float16
BF16_NP = ml_dtypes.bfloat16
F16_NP = np.float16

JK, JQ, JV = 0, 1, 2   # wT column sections: k, q, v


def _build():
    nc = bacc.Bacc("TRN2", target_bir_lowering=False, debug=False,
                   num_devices=N_CORES)

    xT = nc.dram_tensor("xT", [C, T], BF16, kind="ExternalInput").ap()
    wT = nc.dram_tensor("wT", [C, 3 * P], BF16, kind="ExternalInput").ap()
    bkq = nc.dram_tensor("bkq", [P, 2], F32, kind="ExternalInput").ap()
    mask01 = nc.dram_tensor("mask01", [P, 2, P], BF16, kind="ExternalInput").ap()
    pw = nc.dram_tensor("pw", [P, C], BF16, kind="ExternalInput").ap()
    outO = nc.dram_tensor("outO", [T, C], F16, kind="ExternalOutput").ap()

    xT3 = xT.rearrange("(o p) t -> p o t", p=P)      # [128, 8, 2048]
    wT3 = wT.rearrange("(o p) j -> p o j", p=P)      # [128, 8, 384]
    out3 = outO.rearrange("(n p) o -> p n o", p=P)   # [128, 16, 1024]

    with tile.TileContext(nc) as tc:
        with (
            tc.tile_pool(name="sb", bufs=1) as sb,
            tc.tile_pool(name="ps", bufs=1, space="PSUM") as ps,
        ):
            warm = sb.tile([P, W], BF16, name="warm")
            wT_sb = sb.tile([P, NO, 3 * P], BF16, name="wt")
            x_sb = sb.tile([P, NO, T], BF16, name="x")
            bkq_sb = sb.tile([P, 2], F32, name="bkq")
            mask_sb = sb.tile([P, 2, P], BF16, name="mask")
            pw_sb = sb.tile([P, C], BF16, name="pw")
            ones_sb = sb.tile([1, D], BF16, name="ones")
            kT_sb = sb.tile([P, T], BF16, name="kt")
            qT_sb = sb.tile([P, T], BF16, name="qt")
            v_sb = sb.tile([P, NT, 2, D + 1], BF16, name="v")
            e2r = [sb.tile([P, 2, W], BF16, name=f"e2_{i}") for i in range(3)]
            rsr = [sb.tile([1, W], BF16, name=f"rs_{i}") for i in range(2)]
            rrr = [sb.tile([D, W], F32, name=f"rr_{i}") for i in range(2)]
            attnr = [sb.tile([P, W], BF16, name=f"attn_{i}") for i in range(2)]
            out_sb = sb.tile([P, 4, C], F16, name="out")

            scr = [ps.tile([P, W], F32, name=f"sc_{i}") for i in range(2)]
            atr = [ps.tile([P, W], F32, name=f"at_{i}") for i in range(2)]
            qsr = [ps.tile([P, W], F32, name=f"qs_{i}") for i in range(2)]
            prr = [ps.tile([P, W], F32, name=f"pr_{i}") for i in range(2)]

            nc.vector.memset(warm[:], 1.0)
            nc.vector.memset(ones_sb[:], 1.0)
            nc.vector.memset(v_sb[:, :, :, D], 1.0)

            # ---- PE warm-up on the memset tile until the first x chunk
            # lands; the HAM clock gate needs ~3.4us of sustained activity ----
            for _ in range(10):
                nc.tensor.matmul(prr[0][:], lhsT=warm[:, 0:P], rhs=warm[:],
                                 start=True, stop=True)

            # ---- input DMAs: x moves in 1024-col pieces (2KB contiguous
            # lines, full DMA rate) split o-wise across both queues; wT first
            # on sync (needed by the first qkv LDWEIGHTS) ----
            nc.sync.dma_start(wT_sb[:], wT3[:])
            nc.gpsimd.dma_start(x_sb[:, 4:8, 0:2 * W], xT3[:, 4:8, 0:2 * W])
            nc.sync.dma_start(x_sb[:, 0:4, 0:2 * W], xT3[:, 0:4, 0:2 * W])
            nc.gpsimd.dma_start(mask_sb[:], mask01)
            nc.gpsimd.dma_start(bkq_sb[:], bkq)
            nc.gpsimd.dma_start(pw_sb[:], pw)
            nc.sync.dma_start(x_sb[:, 0:4, 2 * W:4 * W], xT3[:, 0:4, 2 * W:4 * W])
            nc.gpsimd.dma_start(x_sb[:, 4:8, 2 * W:4 * W], xT3[:, 4:8, 2 * W:4 * W])

            fill_qkv = deque()
            fill_proj = deque()
            state = {"qs": 0, "pr": 0, "dmaq": 0}

            def qkv_block(b):
                """Queue qkv for token block b (512 tokens) as PE thunks."""
                cols = slice(b * W, (b + 1) * W)
                st = {}

                def kq_mm(sec, dst, bi, o, i):
                    def f():
                        if i == 0:
                            st[sec] = qsr[state["qs"] % 2]
                            state["qs"] += 1
                        nc.tensor.matmul(
                            st[sec][:],
                            lhsT=wT_sb[:, o, sec * P:(sec + 1) * P],
                            rhs=x_sb[:, o, cols],
                            start=(i == 0), stop=(i == NO - 1),
                        )
                        if i == NO - 1:
                            # k evac on DVE, q on ACT (per-partition bias)
                            if sec == JK:
                                nc.vector.tensor_scalar_add(
                                    dst[:, cols], st[sec][:], bkq_sb[:, 0:1])
                            else:
                                nc.scalar.add(dst[:, cols], st[sec][:],
                                              bkq_sb[:, 1:2])
                    return f

                def v_mm(tt, o):
                    def f():
                        if tt == 0 and o == 0:
                            st[JV] = qsr[state["qs"] % 2]
                            state["qs"] += 1
                        st4 = st[JV].rearrange("p (a h d) -> p a h d",
                                               a=4, h=2, d=D)
                        t0 = b * W + tt * P
                        nc.tensor.matmul(
                            st4[:, tt, :, :],
                            lhsT=x_sb[:, o, t0:t0 + P],
                            rhs=wT_sb[:, o, JV * P:(JV + 1) * P],
                            start=(o == 0), stop=(o == NO - 1),
                        )
                        if tt == 3 and o == NO - 1:
                            nc.vector.tensor_copy(
                                out=v_sb[:, 4 * b:4 * b + 4, :, 0:D],
                                in_=st4[:])
                    return f

                # o-order 4..7 first: that x half lands first (gpsimd queue)
                oorder = [4, 5, 6, 7, 0, 1, 2, 3]
                for sec, dst in ((JK, kT_sb), (JQ, qT_sb)):
                    for i, o in enumerate(oorder):
                        fill_qkv.append(kq_mm(sec, dst, sec, o, i))
                for tt in range(4):
                    for o in range(NO):
                        fill_qkv.append(v_mm(tt, o))

            def proj_tt(c, tt, last):
                """Project one token tile of chunk c; evacuate + DMA."""
                def f():
                    a = attnr[c % 2][:, tt * P:(tt + 1) * P]
                    p0 = prr[state["pr"] % 2]
                    p1 = prr[(state["pr"] + 1) % 2]
                    state["pr"] += 2
                    nc.tensor.matmul(p0[:], lhsT=a, rhs=pw_sb[:, 0:W],
                                     start=True, stop=True)
                    nc.tensor.matmul(p1[:], lhsT=a, rhs=pw_sb[:, W:C],
                                     start=True, stop=True)
                    if last:
                        # tail: ACT has no more exps; split the evacuation
                        nc.vector.tensor_copy(out=out_sb[:, tt, 0:W], in_=p0[:])
                        nc.scalar.copy(out_sb[:, tt, W:C], p1[:])
                    else:
                        nc.vector.tensor_copy(out=out_sb[:, tt, 0:W], in_=p0[:])
                        nc.vector.tensor_copy(out=out_sb[:, tt, W:C], in_=p1[:])
                    if tt % 2 == 1:
                        gt = c * 4 + tt
                        src = out_sb[:, tt - 1:tt + 1, :]
                        dst = out3[:, gt - 1:gt + 1, :]
                        if state["dmaq"] % 2 == 0:
                            nc.sync.dma_start(dst, src)
                        else:
                            nc.gpsimd.dma_start(dst, src)
                        state["dmaq"] += 1
                return f

            def pop(dq, n):
                for _ in range(n):
                    if dq:
                        dq.popleft()()

            def popn(n):
                """Boundary padding: qkv fillers first, then reserved proj."""
                for _ in range(n):
                    if fill_qkv:
                        fill_qkv.popleft()()
                    elif fill_proj:
                        fill_proj.popleft()()

            def attention(c):
                t0 = c * W
                nj = 4 * (c + 1)
                keep = 2
                pend = None

                def av(j, tq, wj, e2):
                    off = tq - t0
                    for h in range(2):
                        nc.tensor.matmul(
                            atr[h][0:D + 1, off:W],
                            lhsT=v_sb[:, j, h, :],
                            rhs=e2[:, h, 0:wj],
                            start=(j == 0), stop=(j == nj - 1),
                        )

                for j in range(nj):
                    diag = j >= 4 * c
                    tq = P * j if diag else t0
                    wj = t0 + W - tq
                    for h in range(2):
                        nc.tensor.matmul(
                            scr[h][:, 0:wj],
                            lhsT=kT_sb[h * D:(h + 1) * D, j * P:(j + 1) * P],
                            rhs=qT_sb[h * D:(h + 1) * D, tq:t0 + W],
                            start=True, stop=True,
                        )
                    pop(fill_qkv, 2)
                    if len(fill_proj) > keep:
                        pop(fill_proj, 1)
                    if pend is not None:
                        av(*pend)
                    e2 = e2r[j % 3]
                    if diag:
                        # masked diag block exp'd first; GpSimd applies the
                        # 0/1 mask while ACT exps the rest of the tile
                        for h in range(2):
                            nc.scalar.activation(
                                e2[:, h, 0:P], scr[h][:, 0:P],
                                mybir.ActivationFunctionType.Exp)
                        nc.gpsimd.tensor_mul(out=e2[:, :, 0:P],
                                             in0=e2[:, :, 0:P],
                                             in1=mask_sb[:])
                        if wj > P:
                            for h in range(2):
                                nc.scalar.activation(
                                    e2[:, h, P:wj], scr[h][:, P:wj],
                                    mybir.ActivationFunctionType.Exp)
                    else:
                        for h in range(2):
                            nc.scalar.activation(
                                e2[:, h, 0:wj], scr[h][:, 0:wj],
                                mybir.ActivationFunctionType.Exp)
                    pend = (j, tq, wj, e2)
                av(*pend)

                # ---- boundary: denominator broadcast + normalize, paced by
                # leftover qkv fillers + reserved proj so the PE FIFO stays
                # fed. The last chunk pipelines normalize+proj in two column
                # halves so the tail chain is half as deep. ----
                tail = (c == NG - 1)
                halves = [(0, W // 2), (W // 2, W)] if tail else [(0, W)]
                for ci, (c0, c1) in enumerate(halves):
                    for h in range(2):
                        nc.scalar.copy(rsr[h][:, c0:c1],
                                       atr[h][D:D + 1, c0:c1])
                    popn(2)
                    # borrow the scores banks between chunks (one per head;
                    # matmul psum output must start at partition 0)
                    for h in range(2):
                        nc.tensor.matmul(
                            scr[h][0:D, c0:c1],
                            lhsT=ones_sb[:],
                            rhs=rsr[h][:, c0:c1],
                            start=True, stop=True,
                        )
                    popn(2)
                    for h in range(2):
                        nc.vector.reciprocal_approx_fast(
                            out=rrr[h][:, c0:c1], in_=scr[h][0:D, c0:c1])
                    popn(1)
                    for h in range(2):
                        nc.vector.tensor_mul(
                            out=attnr[c % 2][h * D:(h + 1) * D, c0:c1],
                            in0=atr[h][0:D, c0:c1], in1=rrr[h][:, c0:c1])
                    if tail:
                        for tt in range(c0 // P, c1 // P):
                            proj_tt(c, tt, last=True)()
                while fill_qkv:
                    fill_qkv.popleft()()

            # ---- main schedule ----
            qkv_block(0)
            while fill_qkv:
                fill_qkv.popleft()()
            qkv_block(1)
            for c in range(NG):
                attention(c)
                if c + 2 < NG:
                    qkv_block(c + 2)
                if c < NG - 1:
                    for tt in range(4):
                        fill_proj.append(proj_tt(c, tt, last=False))
            while fill_proj:
                fill_proj.popleft()()

    nc.compile()
    return nc


_NC = None
LAST_RESULT = None


def _get_nc():
    global _NC
    if _NC is None:
        _NC = _build()
    return _NC


def _prep_inputs(x, wqkv_w, wqkv_b, proj_w, proj_b):
    x = np.asarray(x, np.float32)
    wqkv_w = np.asarray(wqkv_w, np.float32)
    wqkv_b = np.asarray(wqkv_b, np.float32)
    proj_w = np.asarray(proj_w, np.float32)

    scale = np.float32(1.0 / np.sqrt(D))  # 0.125 exactly
    xT = np.ascontiguousarray(x.T).astype(BF16_NP)
    mask1 = np.triu(np.ones((P, P), np.float32))
    mask = np.ascontiguousarray(
        np.broadcast_to(mask1[:, None, :], (P, 2, P))).astype(BF16_NP)

    in_maps = []
    for c in range(N_CORES):
        qs = slice(P * c, P * (c + 1))
        ks = slice(C + P * c, C + P * (c + 1))
        vs = slice(2 * C + P * c, 2 * C + P * (c + 1))
        # column order in wT: k, q (pre-scaled), v
        w_c = np.concatenate(
            [wqkv_w[ks], wqkv_w[qs] * scale, wqkv_w[vs]], axis=0)  # [384, 1024]
        b_c = np.stack(
            [wqkv_b[ks], wqkv_b[qs] * scale], axis=1)              # [128, 2]
        in_maps.append({
            "xT": xT,
            "wT": np.ascontiguousarray(w_c.T).astype(BF16_NP),
            "bkq": np.ascontiguousarray(b_c, dtype=np.float32),
            "mask01": mask,
            # proj rows for this core's 128 attn dims -> [128, 1024]
            "pw": np.ascontiguousarray(proj_w[:, qs].T).astype(BF16_NP),
        })
    return in_maps


def kernel(x, wqkv_w, wqkv_b, proj_w, proj_b):
    global LAST_RESULT
    nc = _get_nc()
    in_maps = _prep_inputs(x, wqkv_w, wqkv_b, proj_w, proj_b)
    res = bass_utils.run_bass_kernel_spmd(nc, in_maps,
                                          core_ids=list(range(N_CORES)))
    LAST_RESULT = res
    # unshard: partials are sum-sharded over cores; v-bias and proj bias fold
    # into one host-side vector add (softmax rows sum to 1).
    acc = res.results[0]["outO"].astype(np.float32)
    for c in range(1, N_CORES):
        acc = acc + res.results[c]["outO"].astype(np.float32)
    b_v = np.asarray(wqkv_b, np.float32)[2 * C:3 * C]
    bias = b_v @ np.asarray(proj_w, np.float32).T + np.asarray(proj_b, np.float32)
    return np.ascontiguousarray(acc + bias[None, :]).astype(np.float32)


# revision 16
# speedup vs baseline: 1.0600x; 1.0598x over previous
"""Causal self-attention (T=2048, C=1024, H=16) on 8 Trainium2 NeuronCores.

Tensor-parallel over heads: each core owns 2 heads (wqkv row-shard), computes
qkv + attention for its heads, then multiplies attention rows by the full
projection matrix to produce a PARTIAL token-major output. The host sums the
8 partials and adds (b_v @ proj_w.T + proj_b) once — v-bias commutes with
softmax-normalized attention, so both biases fold into the host-side reduce.

v4 (vs v3 at ~104-107us):
  - no dummy-warmup phase beyond ~8 matmuls: the input DMA order is arranged
    so qkv(0) itself starts ~3us after the framework preamble and doubles as
    the HAM warm-up.
  - transposed proj: lhsT = attn token-tile, rhs = proj rows -> psum is
    [128 tokens, 512 outs] and the output DMA is token-major [2048, 1024]
    fp16 with 2KB contiguous lines; no host transpose.
  - normalize rescheduled off the PE critical path: denominator row -> ACT
    copy -> f32r ones-broadcast matmul (borrowing a scores psum bank) ->
    DVE reciprocal -> DVE multiply, all emitted between the leftover qkv
    fillers so the PE FIFO never stalls at a chunk boundary (the v3 stall
    re-engaged the HAM clock gate: 32us of the run at 1.2GHz).
  - scores per head in separate psum banks (ring of 2); the two heads' K=64
    matmuls occupy disjoint PE row groups and stream concurrently.
  - diag-block causal mask: exp is split so the masked [128,128] block is
    exp'd first and multiplied by the 0/1 mask on GpSimd while ACT exps the
    rest of the tile - the mask latency hides under the remaining exp.
  - psum: 8 banks = scores 2 + attnT accum 2 + qkv state 2 + proj 2.
  - proj psum evacuations split DVE/ACT at the tail; per-2-token-tile output
    DMAs so the final drain waits on ~256KB, not a whole chunk.
"""

from collections import deque

import numpy as np
import ml_dtypes

import concourse.bass as bass
import concourse.mybir as mybir
import concourse.tile as tile
from concourse import bacc
from concourse import bass_utils

T = 2048
C = 1024
H = 16
D = 64
N_CORES = 8
P = 128
W = 512            # query-chunk width
NG = T // W        # 4 chunks
NO = C // P        # 8 contraction subtiles
NT = T // P        # 16 token tiles

F32 = mybir.dt.float32
F32R = mybir.dt.float32r
BF16 = mybir.dt.bfloat16
F16 = mybir.dt.float16
BF16_NP = ml_dtypes.bfloat16
F16_NP = np.float16

JK, JQ, JV = 0, 1, 2   # wT column sections: k, q, v


def _build():
    nc = bacc.Bacc("TRN2", target_bir_lowering=False, debug=False,
                   num_devices=N_CORES)

    xT = nc.dram_tensor("xT", [C, T], BF16, kind="ExternalInput").ap()
    wT = nc.dram_tensor("wT", [C, 3 * P], BF16, kind="ExternalInput").ap()
    bkq = nc.dram_tensor("bkq", [P, 2], F32, kind="ExternalInput").ap()
    mask01 = nc.dram_tensor("mask01", [P, 2, P], BF16, kind="ExternalInput").ap()
    pw = nc.dram_tensor("pw", [P, C], BF16, kind="ExternalInput").ap()
    outO = nc.dram_tensor("outO", [T, C], F16, kind="ExternalOutput").ap()

    xT3 = xT.rearrange("(o p) t -> p o t", p=P)      # [128, 8, 2048]
    wT3 = wT.rearrange("(o p) j -> p o j", p=P)      # [128, 8, 384]
    out3 = outO.rearrange("(n p) o -> p n o", p=P)   # [128, 16, 1024]

    with tile.TileContext(nc) as tc:
        with (
            tc.tile_pool(name="sb", bufs=1) as sb,
            tc.tile_pool(name="ps", bufs=1, space="PSUM") as ps,
        ):
            warm = sb.tile([P, W], BF16, name="warm")
            wT_sb = sb.tile([P, NO, 3 * P], BF16, name="wt")
            x_sb = sb.tile([P, NO, T], BF16, name="x")
            bkq_sb = sb.tile([P, 2], F32, name="bkq")
            mask_sb = sb.tile([P, 2, P], BF16, name="mask")
            pw_sb = sb.tile([P, C], BF16, name="pw")
            ones_sb = sb.tile([1, D], BF16, name="ones")
            kT_sb = sb.tile([P, T], BF16, name="kt")
            qT_sb = sb.tile([P, T], BF16, name="qt")
            v_sb = sb.tile([P, NT, 2, D + 1], BF16, name="v")
            e2r = [sb.tile([P, 2, W], BF16, name=f"e2_{i}") for i in range(4)]
            rsr = [sb.tile([1, W], BF16, name=f"rs_{i}") for i in range(2)]
            rrr = [sb.tile([D, W], F32, name=f"rr_{i}") for i in range(2)]
            attnr = [sb.tile([P, W], BF16, name=f"attn_{i}") for i in range(2)]
            out_sb = sb.tile([P, 4, C], F16, name="out")

            scr = [ps.tile([P, W], F32, name=f"sc_{i}") for i in range(2)]
            atr = [ps.tile([P, W], F32, name=f"at_{i}") for i in range(2)]
            qsr = [ps.tile([P, W], F32, name=f"qs_{i}") for i in range(2)]
            prr = [ps.tile([P, W], F32, name=f"pr_{i}") for i in range(2)]

            nc.vector.memset(warm[:], 1.0)
            nc.vector.memset(ones_sb[:], 1.0)
            nc.vector.memset(v_sb[:, :, :, D], 1.0)

            # ---- PE warm-up on the memset tile until the first x chunk
            # lands; the HAM clock gate needs ~3.4us of sustained activity ----
            for _ in range(10):
                nc.tensor.matmul(prr[0][:], lhsT=warm[:, 0:P], rhs=warm[:],
                                 start=True, stop=True)

            # ---- input DMAs: x moves in 1024-col pieces (2KB contiguous
            # lines, full DMA rate) split o-wise across both queues; wT first
            # on sync (needed by the first qkv LDWEIGHTS) ----
            nc.sync.dma_start(wT_sb[:], wT3[:])
            nc.gpsimd.dma_start(x_sb[:, 4:8, 0:2 * W], xT3[:, 4:8, 0:2 * W])
            nc.sync.dma_start(x_sb[:, 0:4, 0:2 * W], xT3[:, 0:4, 0:2 * W])
            nc.gpsimd.dma_start(mask_sb[:], mask01)
            nc.gpsimd.dma_start(bkq_sb[:], bkq)
            nc.gpsimd.dma_start(pw_sb[:], pw)
            nc.sync.dma_start(x_sb[:, 0:4, 2 * W:4 * W], xT3[:, 0:4, 2 * W:4 * W])
            nc.gpsimd.dma_start(x_sb[:, 4:8, 2 * W:4 * W], xT3[:, 4:8, 2 * W:4 * W])

            fill_qkv = deque()
            fill_proj = deque()
            state = {"qs": 0, "pr": 0, "dmaq": 0}

            def qkv_block(b):
                """Queue qkv for token block b (512 tokens) as PE thunks."""
                cols = slice(b * W, (b + 1) * W)
                st = {}

                def kq_mm(sec, dst, bi, o, i):
                    def f():
                        if i == 0:
                            st[sec] = qsr[state["qs"] % 2]
                            state["qs"] += 1
                        nc.tensor.matmul(
                            st[sec][:],
                            lhsT=wT_sb[:, o, sec * P:(sec + 1) * P],
                            rhs=x_sb[:, o, cols],
                            start=(i == 0), stop=(i == NO - 1),
                        )
                        if i == NO - 1:
                            # k evac on DVE, q on ACT (per-partition bias)
                            if sec == JK:
                                nc.vector.tensor_scalar_add(
                                    dst[:, cols], st[sec][:], bkq_sb[:, 0:1])
                            else:
                                nc.scalar.add(dst[:, cols], st[sec][:],
                                              bkq_sb[:, 1:2])
                    return f

                def v_mm(tt, o):
                    def f():
                        if tt == 0 and o == 0:
                            st[JV] = qsr[state["qs"] % 2]
                            state["qs"] += 1
                        st4 = st[JV].rearrange("p (a h d) -> p a h d",
                                               a=4, h=2, d=D)
                        t0 = b * W + tt * P
                        nc.tensor.matmul(
                            st4[:, tt, :, :],
                            lhsT=x_sb[:, o, t0:t0 + P],
                            rhs=wT_sb[:, o, JV * P:(JV + 1) * P],
                            start=(o == 0), stop=(o == NO - 1),
                        )
                        if tt == 3 and o == NO - 1:
                            nc.vector.tensor_copy(
                                out=v_sb[:, 4 * b:4 * b + 4, :, 0:D],
                                in_=st4[:])
                    return f

                # o-order 4..7 first: that x half lands first (gpsimd queue)
                oorder = [4, 5, 6, 7, 0, 1, 2, 3]
                for sec, dst in ((JK, kT_sb), (JQ, qT_sb)):
                    for i, o in enumerate(oorder):
                        fill_qkv.append(kq_mm(sec, dst, sec, o, i))
                for tt in range(4):
                    for o in range(NO):
                        fill_qkv.append(v_mm(tt, o))

            def proj_tt(c, tt, last):
                """Project one token tile of chunk c; evacuate + DMA."""
                def f():
                    a = attnr[c % 2][:, tt * P:(tt + 1) * P]
                    p0 = prr[state["pr"] % 2]
                    p1 = prr[(state["pr"] + 1) % 2]
                    state["pr"] += 2
                    nc.tensor.matmul(p0[:], lhsT=a, rhs=pw_sb[:, 0:W],
                                     start=True, stop=True)
                    nc.tensor.matmul(p1[:], lhsT=a, rhs=pw_sb[:, W:C],
                                     start=True, stop=True)
                    if last:
                        # tail: ACT has no more exps; split the evacuation
                        nc.vector.tensor_copy(out=out_sb[:, tt, 0:W], in_=p0[:])
                        nc.scalar.copy(out_sb[:, tt, W:C], p1[:])
                    else:
                        nc.vector.tensor_copy(out=out_sb[:, tt, 0:W], in_=p0[:])
                        nc.vector.tensor_copy(out=out_sb[:, tt, W:C], in_=p1[:])
                    if tt % 2 == 1:
                        gt = c * 4 + tt
                        src = out_sb[:, tt - 1:tt + 1, :]
                        dst = out3[:, gt - 1:gt + 1, :]
                        if state["dmaq"] % 2 == 0:
                            nc.sync.dma_start(dst, src)
                        else:
                            nc.gpsimd.dma_start(dst, src)
                        state["dmaq"] += 1
                return f

            def pop(dq, n):
                for _ in range(n):
                    if dq:
                        dq.popleft()()

            def popn(n):
                """Boundary padding: qkv fillers first, then reserved proj."""
                for _ in range(n):
                    if fill_qkv:
                        fill_qkv.popleft()()
                    elif fill_proj:
                        fill_proj.popleft()()

            def attention(c):
                t0 = c * W
                nj = 4 * (c + 1)
                keep = 2

                def av(j, tq, wj, e2, first, last):
                    off = tq - t0
                    for h in range(2):
                        nc.tensor.matmul(
                            atr[h][0:D + 1, off:W],
                            lhsT=v_sb[:, j, h, :],
                            rhs=e2[:, h, 0:wj],
                            start=first, stop=last,
                        )

                # diag j-tiles first: their exp->mask chains hide under the
                # long full-width j's that follow instead of starving the PE
                # at the chunk tail
                order = list(range(4 * c, nj)) + list(range(0, 4 * c))
                pends = deque()
                for idx, j in enumerate(order):
                    diag = j >= 4 * c
                    tq = P * j if diag else t0
                    wj = t0 + W - tq
                    for h in range(2):
                        nc.tensor.matmul(
                            scr[h][:, 0:wj],
                            lhsT=kT_sb[h * D:(h + 1) * D, j * P:(j + 1) * P],
                            rhs=qT_sb[h * D:(h + 1) * D, tq:t0 + W],
                            start=True, stop=True,
                        )
                    pop(fill_qkv, 2)
                    if len(fill_proj) > keep:
                        pop(fill_proj, 1)
                    if len(pends) >= 2:
                        av(*pends.popleft())
                    e2 = e2r[idx % 4]
                    if diag:
                        # masked diag block per head: exp then 0/1 mask on
                        # GpSimd, chained per head so av_h0 never waits on
                        # the h1 exp
                        for h in range(2):
                            nc.scalar.activation(
                                e2[:, h, 0:P], scr[h][:, 0:P],
                                mybir.ActivationFunctionType.Exp)
                            nc.gpsimd.tensor_mul(out=e2[:, h, 0:P],
                                                 in0=e2[:, h, 0:P],
                                                 in1=mask_sb[:, h, :])
                        if wj > P:
                            for h in range(2):
                                nc.scalar.activation(
                                    e2[:, h, P:wj], scr[h][:, P:wj],
                                    mybir.ActivationFunctionType.Exp)
                    else:
                        for h in range(2):
                            nc.scalar.activation(
                                e2[:, h, 0:wj], scr[h][:, 0:wj],
                                mybir.ActivationFunctionType.Exp)
                    pends.append((j, tq, wj, e2, idx == 0, idx == nj - 1))
                while pends:
                    av(*pends.popleft())

                # ---- boundary: denominator broadcast + normalize, paced by
                # leftover qkv fillers + reserved proj so the PE FIFO stays
                # fed. The last chunk pipelines normalize+proj in two column
                # halves so the tail chain is half as deep. ----
                tail = (c == NG - 1)
                halves = [(0, W // 2), (W // 2, W)] if tail else [(0, W)]
                for ci, (c0, c1) in enumerate(halves):
                    for h in range(2):
                        nc.scalar.copy(rsr[h][:, c0:c1],
                                       atr[h][D:D + 1, c0:c1])
                    popn(2)
                    # borrow the scores banks between chunks (one per head;
                    # matmul psum output must start at partition 0)
                    for h in range(2):
                        nc.tensor.matmul(
                            scr[h][0:D, c0:c1],
                            lhsT=ones_sb[:],
                            rhs=rsr[h][:, c0:c1],
                            start=True, stop=True,
                        )
                    popn(2)
                    for h in range(2):
                        nc.vector.reciprocal_approx_fast(
                            out=rrr[h][:, c0:c1], in_=scr[h][0:D, c0:c1])
                    popn(1)
                    for h in range(2):
                        nc.vector.tensor_mul(
                            out=attnr[c % 2][h * D:(h + 1) * D, c0:c1],
                            in0=atr[h][0:D, c0:c1], in1=rrr[h][:, c0:c1])
                    if tail:
                        for tt in range(c0 // P, c1 // P):
                            proj_tt(c, tt, last=True)()
                while fill_qkv:
                    fill_qkv.popleft()()

            # ---- main schedule ----
            qkv_block(0)
            while fill_qkv:
                fill_qkv.popleft()()
            qkv_block(1)
            for c in range(NG):
                attention(c)
                if c + 2 < NG:
                    qkv_block(c + 2)
                if c < NG - 1:
                    for tt in range(4):
                        fill_proj.append(proj_tt(c, tt, last=False))
            while fill_proj:
                fill_proj.popleft()()

    nc.compile()
    return nc


_NC = None
LAST_RESULT = None


def _get_nc():
    global _NC
    if _NC is None:
        _NC = _build()
    return _NC


def _prep_inputs(x, wqkv_w, wqkv_b, proj_w, proj_b):
    x = np.asarray(x, np.float32)
    wqkv_w = np.asarray(wqkv_w, np.float32)
    wqkv_b = np.asarray(wqkv_b, np.float32)
    proj_w = np.asarray(proj_w, np.float32)

    scale = np.float32(1.0 / np.sqrt(D))  # 0.125 exactly
    xT = np.ascontiguousarray(x.T).astype(BF16_NP)
    mask1 = np.triu(np.ones((P, P), np.float32))
    mask = np.ascontiguousarray(
        np.broadcast_to(mask1[:, None, :], (P, 2, P))).astype(BF16_NP)

    in_maps = []
    for c in range(N_CORES):
        qs = slice(P * c, P * (c + 1))
        ks = slice(C + P * c, C + P * (c + 1))
        vs = slice(2 * C + P * c, 2 * C + P * (c + 1))
        # column order in wT: k, q (pre-scaled), v
        w_c = np.concatenate(
            [wqkv_w[ks], wqkv_w[qs] * scale, wqkv_w[vs]], axis=0)  # [384, 1024]
        b_c = np.stack(
            [wqkv_b[ks], wqkv_b[qs] * scale], axis=1)              # [128, 2]
        in_maps.append({
            "xT": xT,
            "wT": np.ascontiguousarray(w_c.T).astype(BF16_NP),
            "bkq": np.ascontiguousarray(b_c, dtype=np.float32),
            "mask01": mask,
            # proj rows for this core's 128 attn dims -> [128, 1024]
            "pw": np.ascontiguousarray(proj_w[:, qs].T).astype(BF16_NP),
        })
    return in_maps


def kernel(x, wqkv_w, wqkv_b, proj_w, proj_b):
    global LAST_RESULT
    nc = _get_nc()
    in_maps = _prep_inputs(x, wqkv_w, wqkv_b, proj_w, proj_b)
    res = bass_utils.run_bass_kernel_spmd(nc, in_maps,
                                          core_ids=list(range(N_CORES)))
    LAST_RESULT = res
    # unshard: partials are sum-sharded over cores; v-bias and proj bias fold
    # into one host-side vector add (softmax rows sum to 1).
    acc = res.results[0]["outO"].astype(np.float32)
    for c in range(1, N_CORES):
        acc = acc + res.results[c]["outO"].astype(np.float32)
    b_v = np.asarray(wqkv_b, np.float32)[2 * C:3 * C]
    bias = b_v @ np.asarray(proj_w, np.float32).T + np.asarray(proj_b, np.float32)
    return np.ascontiguousarray(acc + bias[None, :]).astype(np.float32)
